# revision 8
# baseline (speedup 1.0000x reference)
"""Masked multi-head attention on 8 NeuronCores (faithful torch raw-view semantics).

The reference reshapes (bs, sql, nh*edim) -> (bs, nh, sql, edim) as a RAW VIEW:
head h's length-1024 pseudo-sequence is built from x rows 128h..128h+127 (each
row contributes 8 pseudo-positions, one per 256-col block of the projection),
and output rows 128h..128h+128 depend only on head h. So the work splits into
32 independent (batch, head) pairs -> 4 per core, no cross-core reduction.

Per (b, h): Q/K/V = x[b,128h:128h+128] @ w{q,k,v}.T + b (full 2048-wide), viewed
as (1024, 256) row-major. Pseudo-positions are kept in NATURAL order s' = 8r+cb
on every axis (columns of Q^T/K^T/V^T/oh are s'-major, written with stride-8
access patterns from the projection epilogues). That makes the causal mask
block-triangular: per (q-half, key-block) tile we either skip it entirely
(fully masked), run it unmasked, or run a narrowed matmul + one shared
triangular mask on the live column suffix. V is rotated to key-major partitions
with XBAR DMA transposes (16-bit), so P@V consumes s'-ordered key blocks.

All matmul operands are bf16 (same PE rate as fp32r here, half the DMA/SBUF);
accumulation stays fp32 in PSUM. Q weights/bias pre-scaled by 1/16. Epilogues
are spread across engines: P1 bias-pack on Act (Identity+bias), P2 bias-pack
and normalization on DVE, mask adds on GpSimd/Pool, exp on Act.
"""

import sys

sys.path.insert(0, "/opt/trn_rl_repo")

import ml_dtypes
import numpy as np

from concourse import bacc, mybir
from concourse.tile import TileContext
from concourse.bass_utils import run_bass_kernel_spmd

EDIM = 256
BS = 4
SQL = 1024
HPC = 4           # heads per core
NCORES = 8
FDT = mybir.dt.float32
BDT = mybir.dt.bfloat16
RDT = mybir.dt.float32r
NEG = -1.0e30

_cache = {}


def _build():
    nc = bacc.Bacc(dynamic_dma_scratch_size=512)

    xt0 = nc.declare_dram_parameter("xt0", [128, 512], RDT, isOutput=False)
    xt1 = nc.declare_dram_parameter("xt1", [128, 512], RDT, isOutput=False)
    wqk0 = nc.declare_dram_parameter("wqk0", [128, 4096], RDT, isOutput=False)
    wqk1 = nc.declare_dram_parameter("wqk1", [128, 4096], RDT, isOutput=False)
    wv0 = nc.declare_dram_parameter("wv0", [128, 2048], RDT, isOutput=False)
    wv1 = nc.declare_dram_parameter("wv1", [128, 2048], RDT, isOutput=False)
    bqk = nc.declare_dram_parameter("bqk", [128, 32], FDT, isOutput=False)
    bvp = nc.declare_dram_parameter("bvp", [128, 16], FDT, isOutput=False)
    mask = nc.declare_dram_parameter("mask", [128, 512], BDT, isOutput=False)
    wot = nc.declare_dram_parameter("wot", [128, 4096], RDT, isOutput=False)
    onc = nc.declare_dram_parameter("onc", [128, 128], BDT, isOutput=False)
    y = nc.declare_dram_parameter("y", [512, 256], FDT, isOutput=True)

    with TileContext(nc) as tc:
        with (
            tc.tile_pool(name="const", bufs=1) as cpool,
            tc.tile_pool(name="w4k", bufs=3) as wqpool,
            tc.tile_pool(name="v2k", bufs=4) as vpool,
            tc.tile_pool(name="vst", bufs=1) as vstpool,
            tc.tile_pool(name="qk4k", bufs=4) as qkpool,
            tc.tile_pool(name="work", bufs=2) as wpool,
            tc.tile_pool(name="ps_a", bufs=3, space="PSUM") as ps_a,
            tc.tile_pool(name="ps_o", bufs=1, space="PSUM") as ps_o,
            tc.tile_pool(name="ps_se", bufs=1, space="PSUM") as ps_se,
        ):
            def load(pool, name, src, shape, dt=FDT, tag=None):
                t = pool.tile(shape, dt, tag=tag or name, name=name)
                nc.sync.dma_start(out=t[:, :], in_=src)
                return t

            def mm(out, lhsT, rhs, **kw):
                nc.tensor.matmul(out, lhsT, rhs, **kw)

            xt_sb = [load(cpool, "xt0", xt0[:, :], [128, 512], dt=RDT),
                     load(cpool, "xt1", xt1[:, :], [128, 512], dt=RDT)]
            bqk_sb = load(cpool, "bqk", bqk[:, :], [128, 32])
            bvp_sb = load(cpool, "bvp", bvp[:, :], [128, 16])
            wqk_srcs = [wqk0, wqk1]
            wqk_sb = []
            for d in range(2):
                t = wqpool.tile([128, 4096], RDT, tag="w4k", name=f"wqk{d}")
                for ch in range(4):
                    nc.sync.dma_start(
                        out=t[:, ch * 1024:(ch + 1) * 1024],
                        in_=wqk_srcs[d][:, ch * 1024:(ch + 1) * 1024],
                    )
                wqk_sb.append(t)
            wv_sb = [load(vpool, "wv0", wv0[:, :], [128, 2048], dt=RDT,
                          tag="v2k"),
                     load(vpool, "wv1", wv1[:, :], [128, 2048], dt=RDT,
                          tag="v2k")]
            mask_sb = load(cpool, "mask", mask[:, :], [128, 512], dt=BDT)
            ones_c = load(cpool, "ones_c", onc[:, :], [128, 128], dt=BDT)

            # d-major Q^T/K^T, s'-major cols: col = hl*1024 + 8r + cb
            qt = [qkpool.tile([128, 4096], BDT, tag="qk4k", name=f"qt{c}")
                  for c in range(2)]
            kt = [qkpool.tile([128, 4096], BDT, tag="qk4k", name=f"kt{c}")
                  for c in range(2)]
            # V^T d-major (vcol partitions), s'-major cols; then rotated to
            # key-major partitions per 128-block via XBAR DMA transpose.
            vT = [vpool.tile([128, 4096], BDT, tag="v2k", name=f"vT{c}")
                  for c in range(2)]
            v_st = [[vstpool.tile([128, 128], BDT, tag=f"vst{i}_{c}",
                                  name=f"vst{i}_{c}") for c in range(2)]
                    for i in range(32)]

            def spread(dst_tile, ps, cb, bias, engine):
                """bias-add ps ([128, 4 heads * 128 r]) and scatter into
                s'-major columns (stride 8, offset cb) of dst_tile."""
                ov = dst_tile.rearrange("p (h r e) -> p h r e", h=4, e=8)
                iv = ps.rearrange("p (h r e) -> p h r e", h=4, e=1)
                if engine == "act":
                    nc.scalar.activation(
                        ov[:, :, :, cb:cb + 1], iv[:, :, :, :],
                        mybir.ActivationFunctionType.Identity,
                        bias=bias,
                    )
                else:
                    nc.vector.tensor_scalar_add(
                        out=ov[:, :, :, cb:cb + 1], in0=iv[:, :, :, :],
                        scalar1=bias,
                    )

            # ---- P1: Q^T/K^T projections, d-major (all 4 heads at once) ----
            for s in range(2):          # 0 = Q, 1 = K
                dst = qt if s == 0 else kt
                for cb in range(8):
                    for c in range(2):
                        ps = ps_a.tile([128, 512], FDT, tag="proj", name="proj")
                        for d in range(2):
                            mm(
                                ps[:, :],
                                wqk_sb[d][:, s * 2048 + cb * 256 + c * 128:
                                          s * 2048 + cb * 256 + c * 128 + 128],
                                xt_sb[d][:, :],
                                start=(d == 0), stop=(d == 1),
                            )
                        bi = s * 16 + cb * 2 + c
                        spread(dst[c], ps, cb, bqk_sb[:, bi:bi + 1], "act")

            # ---- P2: V^T projection, d-major, s'-major cols ----
            for cb in range(8):
                for c in range(2):
                    ps = ps_a.tile([128, 512], FDT, tag="proj", name="vproj")
                    for d in range(2):
                        mm(
                            ps[:, :],
                            wv_sb[d][:, cb * 256 + c * 128:
                                     cb * 256 + c * 128 + 128],
                            xt_sb[d][:, :],
                            start=(d == 0), stop=(d == 1),
                        )
                    spread(vT[c], ps, cb, bvp_sb[:, cb * 2 + c:cb * 2 + c + 1],
                           "dve")

            # rotate V to key-major partitions: v_st[hl*8+t] = V[s' block t]^T
            for hl in range(HPC):
                for t in range(8):
                    for c in range(2):
                        nc.sync.dma_start_transpose(
                            out=v_st[hl * 8 + t][c][:, :],
                            in_=vT[c][:, hl * 1024 + t * 128:
                                      hl * 1024 + t * 128 + 128],
                        )

            tc.strict_bb_all_engine_barrier()

            # output of attention, normalized, s'-major cols
            oh = [wqpool.tile([128, 4096], RDT, tag="w4k", name=f"oh{c}")
                  for c in range(2)]
            wot_sb = load(wqpool, "wot", wot[:, :], [128, 4096], dt=RDT,
                          tag="w4k")

            # ---- P3: attention per head; causal skip in s' order ----
            for hl in range(HPC):
                for qj in range(2):
                    last = 4 * qj + 3
                    po = [ps_o.tile([128, 512], FDT, tag="po", name=f"po{c}",
                                    bufs=3) for c in range(2)]
                    se = ps_se.tile([128, 512], FDT, tag="se", name="se",
                                    bufs=2)
                    for ki in range(last + 1):
                        dlt = ki - 4 * qj
                        off = 128 * dlt if dlt > 0 else 0
                        sp = ps_a.tile([128, 512], FDT, tag="proj",
                                       name="score")
                        for c in range(2):
                            mm(
                                sp[:, off:],
                                kt[c][:, hl * 1024 + ki * 128:
                                      hl * 1024 + ki * 128 + 128],
                                qt[c][:, hl * 1024 + qj * 512 + off:
                                      hl * 1024 + qj * 512 + 512],
                                start=(c == 0), stop=(c == 1),
                            )
                        pt = wpool.tile([128, 512], BDT, tag="pt", name="pt",
                                        bufs=6)
                        nc.scalar.activation(
                            pt[:, off:], sp[:, off:],
                            mybir.ActivationFunctionType.Exp,
                        )
                        if dlt >= 0:
                            if off:
                                nc.gpsimd.memset(pt[:, :off], 0.0)
                            # zero the causally-masked entries (0/1 mask)
                            nc.gpsimd.tensor_mul(
                                out=pt[:, off:], in0=pt[:, off:],
                                in1=mask_sb[:, :512 - off],
                            )
                        for c in range(2):
                            mm(
                                po[c][:, :],
                                v_st[hl * 8 + ki][c][:, :],
                                pt[:, :],
                                start=(ki == 0), stop=(ki == last),
                            )
                        mm(
                            se[:, :], ones_c[:, :], pt[:, :],
                            start=(ki == 0), stop=(ki == last),
                        )
                    rc = wpool.tile([128, 512], FDT, tag="rc", name="rc",
                                    bufs=3)
                    nc.vector.reciprocal(out=rc[:, :], in_=se[:, :])
                    for c in range(2):
                        nc.vector.tensor_mul(
                            out=oh[c][:, hl * 1024 + qj * 512:
                                      hl * 1024 + qj * 512 + 512],
                            in0=po[c][:, :], in1=rc[:, :],
                        )

            # ---- P4: output projection per head (y rows are per-head!) ----
            for hl in range(HPC):
                yp = ps_a.tile([128, 512], FDT, tag="proj", name="yproj")
                for cb in range(8):
                    for c in range(2):
                        j = 2 * cb + c
                        ohv = oh[c].rearrange("p (h r e) -> p h r e",
                                              h=4, e=8)
                        mm(
                            yp[:, 0:256],
                            ohv[:, hl:hl + 1, :, cb:cb + 1],
                            wot_sb[:, j * 256:(j + 1) * 256],
                            start=(j == 0), stop=(j == 15),
                        )
                ys = wpool.tile([128, 256], FDT, tag="ys", name="ys")
                nc.vector.tensor_copy(out=ys[:, :], in_=yp[:, 0:256])
                nc.sync.dma_start(
                    out=y[hl * 128:(hl + 1) * 128, :], in_=ys[:, :]
                )
    nc.finalize()
    return nc


def _prep_inputs(x, w_attn, b_attn, w_out):
    bf16 = ml_dtypes.bfloat16
    # single triangular mask: live col suffix j' of a diagonal tile obeys
    # (key partition p) <= j'
    rk = np.arange(128)
    rq = np.arange(512)
    mask_arr = (rk[:, None] <= rq[None, :]).astype(bf16)  # 1.0 live, 0.0 dead

    wqk = np.ascontiguousarray(
        np.concatenate([w_attn[0:2048] / 16.0, w_attn[2048:4096]]).T
    )  # (256, 4096)
    wvt = np.ascontiguousarray(w_attn[4096:6144].T)  # (256, 2048)
    bqk_arr = np.ascontiguousarray(
        np.concatenate([b_attn[0:2048] / 16.0, b_attn[2048:4096]])
        .reshape(32, 128).T
    )  # (128, 32)
    bvp_arr = np.ascontiguousarray(
        b_attn[4096:6144].reshape(16, 128).T
    ).astype(np.float32)  # (128, 16): col cb*2+c, partition dd
    wot_arr = np.ascontiguousarray(
        w_out.T.reshape(16, 128, 256).transpose(1, 0, 2).reshape(128, 4096)
    )

    in_maps = []
    for c in range(NCORES):
        b, g = divmod(c, 2)
        xt = np.ascontiguousarray(x[b, 512 * g:512 * (g + 1)].T)
        in_maps.append({
            "xt0": np.ascontiguousarray(xt[:128]),
            "xt1": np.ascontiguousarray(xt[128:]),
            "wqk0": np.ascontiguousarray(wqk[:128]),
            "wqk1": np.ascontiguousarray(wqk[128:]),
            "wv0": np.ascontiguousarray(wvt[:128]),
            "wv1": np.ascontiguousarray(wvt[128:]),
            "bqk": bqk_arr.astype(np.float32),
            "bvp": bvp_arr,
            "mask": mask_arr,
            "wot": wot_arr.astype(np.float32),
            "onc": np.ones((128, 128), bf16),
        })
    return in_maps


def kernel(x, w_attn, b_attn, w_out, b_out):
    x = np.asarray(x, dtype=np.float32)
    w_attn = np.asarray(w_attn, dtype=np.float32)
    b_attn = np.asarray(b_attn, dtype=np.float32)
    w_out = np.asarray(w_out, dtype=np.float32)
    b_out = np.asarray(b_out, dtype=np.float32)

    if "nc" not in _cache:
        _cache["nc"] = _build()
    nc = _cache["nc"]

    in_maps = _prep_inputs(x, w_attn, b_attn, w_out)
    res = run_bass_kernel_spmd(nc, in_maps, list(range(NCORES))).results

    out = np.empty((BS, SQL, EDIM), dtype=np.float32)
    for c in range(NCORES):
        b, g = divmod(c, 2)
        out[b, 512 * g:512 * (g + 1)] = res[c]["y"]
    out += b_out
    return out


# revision 10
# speedup vs baseline: 1.3706x; 1.3706x over previous
"""Masked multi-head attention on 8 NeuronCores (faithful torch raw-view semantics).

The reference reshapes (bs, sql, nh*edim) -> (bs, nh, sql, edim) as a RAW VIEW:
head h's length-1024 pseudo-sequence is built from x rows 128h..128h+127 (each
row contributes 8 pseudo-positions, one per 256-col block of the projection),
and output rows 128h..128h+128 depend only on head h. So the work splits into
32 independent (batch, head) pairs -> 4 per core, no cross-core reduction.

Per (b, h): Q/K/V = x[b,128h:128h+128] @ w{q,k,v}.T + b (full 2048-wide), viewed
as (1024, 256) row-major. Pseudo-positions are kept in NATURAL order s' = 8r+cb
on every axis (columns of Q^T/K^T/V^T/oh are s'-major, written with stride-8
access patterns from the projection epilogues). That makes the causal mask
block-triangular: per (q-half, key-block) tile we either skip it entirely
(fully masked), run it unmasked, or run a narrowed matmul + one shared
triangular mask on the live column suffix. V is rotated to key-major partitions
with XBAR DMA transposes (16-bit), so P@V consumes s'-ordered key blocks.

All matmul operands are bf16 (same PE rate as fp32r here, half the DMA/SBUF);
accumulation stays fp32 in PSUM. Q weights/bias pre-scaled by 1/16. Epilogues
are spread across engines: P1 bias-pack on Act (Identity+bias), P2 bias-pack
and normalization on DVE, mask adds on GpSimd/Pool, exp on Act.
"""

import sys

sys.path.insert(0, "/opt/trn_rl_repo")

import ml_dtypes
import numpy as np

from concourse import bacc, mybir
from concourse.tile import TileContext
from concourse.bass_utils import run_bass_kernel_spmd
from concourse.bass import _add_dep_helper


def _dep(a, b, why):
    if a is not None and b is not None:
        _add_dep_helper(a.ins, b.ins, sync=True, reason=why)

EDIM = 256
BS = 4
SQL = 1024
HPC = 4           # heads per core
NCORES = 8
FDT = mybir.dt.float32
BDT = mybir.dt.bfloat16
RDT = mybir.dt.float32r
NEG = -1.0e30

_cache = {}


def _build():
    nc = bacc.Bacc(dynamic_dma_scratch_size=512)

    xt0 = nc.declare_dram_parameter("xt0", [128, 512], RDT, isOutput=False)
    xt1 = nc.declare_dram_parameter("xt1", [128, 512], RDT, isOutput=False)
    wqk0 = nc.declare_dram_parameter("wqk0", [128, 4096], RDT, isOutput=False)
    wqk1 = nc.declare_dram_parameter("wqk1", [128, 4096], RDT, isOutput=False)
    wv0 = nc.declare_dram_parameter("wv0", [128, 2048], RDT, isOutput=False)
    wv1 = nc.declare_dram_parameter("wv1", [128, 2048], RDT, isOutput=False)
    bqk = nc.declare_dram_parameter("bqk", [128, 32], FDT, isOutput=False)
    bvp = nc.declare_dram_parameter("bvp", [128, 16], FDT, isOutput=False)
    mask = nc.declare_dram_parameter("mask", [128, 512], BDT, isOutput=False)
    wot = nc.declare_dram_parameter("wot", [128, 4096], RDT, isOutput=False)
    onc = nc.declare_dram_parameter("onc", [128, 128], BDT, isOutput=False)
    y = nc.declare_dram_parameter("y", [512, 256], FDT, isOutput=True)

    with TileContext(nc) as tc:
        with (
            tc.tile_pool(name="const", bufs=1) as cpool,
            tc.tile_pool(name="w4k", bufs=3) as wqpool,
            tc.tile_pool(name="v2k", bufs=4) as vpool,
            tc.tile_pool(name="vst", bufs=1) as vstpool,
            tc.tile_pool(name="qk4k", bufs=4) as qkpool,
            tc.tile_pool(name="work", bufs=2) as wpool,
            tc.tile_pool(name="ps_a", bufs=3, space="PSUM") as ps_a,
            tc.tile_pool(name="ps_o", bufs=1, space="PSUM") as ps_o,
            tc.tile_pool(name="ps_se", bufs=1, space="PSUM") as ps_se,
        ):
            def load(pool, name, src, shape, dt=FDT, tag=None):
                t = pool.tile(shape, dt, tag=tag or name, name=name)
                nc.sync.dma_start(out=t[:, :], in_=src)
                return t

            def mm(out, lhsT, rhs, **kw):
                return nc.tensor.matmul(out, lhsT, rhs, **kw)

            xt_sb = [load(cpool, "xt0", xt0[:, :], [128, 512], dt=RDT),
                     load(cpool, "xt1", xt1[:, :], [128, 512], dt=RDT)]
            bqk_sb = load(cpool, "bqk", bqk[:, :], [128, 32])
            bvp_sb = load(cpool, "bvp", bvp[:, :], [128, 16])
            wqk_srcs = [wqk0, wqk1]
            wqk_sb = []
            for d in range(2):
                t = wqpool.tile([128, 4096], RDT, tag="w4k", name=f"wqk{d}")
                for ch in range(4):
                    nc.sync.dma_start(
                        out=t[:, ch * 1024:(ch + 1) * 1024],
                        in_=wqk_srcs[d][:, ch * 1024:(ch + 1) * 1024],
                    )
                wqk_sb.append(t)
            wv_sb = [load(vpool, "wv0", wv0[:, :], [128, 2048], dt=RDT,
                          tag="v2k"),
                     load(vpool, "wv1", wv1[:, :], [128, 2048], dt=RDT,
                          tag="v2k")]
            mask_sb = load(cpool, "mask", mask[:, :], [128, 512], dt=BDT)
            ones_c = load(cpool, "ones_c", onc[:, :], [128, 128], dt=BDT)

            # d-major Q^T/K^T, s'-major cols: col = hl*1024 + 8r + cb
            qt = [qkpool.tile([128, 4096], BDT, tag="qk4k", name=f"qt{c}")
                  for c in range(2)]
            kt = [qkpool.tile([128, 4096], BDT, tag="qk4k", name=f"kt{c}")
                  for c in range(2)]
            # V^T d-major (vcol partitions), s'-major cols; then rotated to
            # key-major partitions per 128-block via XBAR DMA transpose.
            vT = [vpool.tile([128, 4096], BDT, tag="v2k", name=f"vT{c}")
                  for c in range(2)]
            v_st = [[vstpool.tile([128, 128], BDT, tag=f"vst{i}_{c}",
                                  name=f"vst{i}_{c}") for c in range(2)]
                    for i in range(32)]

            def spread(dst_tile, ps, cb, bias, engine):
                """bias-add ps ([128, 4 heads * 128 r]) and scatter into
                s'-major columns (stride 8, offset cb) of dst_tile."""
                ov = dst_tile.rearrange("p (h r e) -> p h r e", h=4, e=8)
                iv = ps.rearrange("p (h r e) -> p h r e", h=4, e=1)
                if engine == "act":
                    return nc.scalar.activation(
                        ov[:, :, :, cb:cb + 1], iv[:, :, :, :],
                        mybir.ActivationFunctionType.Identity,
                        bias=bias,
                    )
                return nc.vector.tensor_scalar_add(
                    out=ov[:, :, :, cb:cb + 1], in0=iv[:, :, :, :],
                    scalar1=bias,
                )

            # ---- P1: Q^T/K^T projections, d-major (all 4 heads at once) ----
            for s in range(2):          # 0 = Q, 1 = K
                dst = qt if s == 0 else kt
                for cb in range(8):
                    for c in range(2):
                        ps = ps_a.tile([128, 512], FDT, tag="proj", name="proj")
                        for d in range(2):
                            mm(
                                ps[:, :],
                                wqk_sb[d][:, s * 2048 + cb * 256 + c * 128:
                                          s * 2048 + cb * 256 + c * 128 + 128],
                                xt_sb[d][:, :],
                                start=(d == 0), stop=(d == 1),
                            )
                        bi = s * 16 + cb * 2 + c
                        spread(dst[c], ps, cb, bqk_sb[:, bi:bi + 1], "act")

            vT_eps = [[], []]
            # ---- P2: V^T projection, d-major, s'-major cols ----
            for cb in range(8):
                for c in range(2):
                    ps = ps_a.tile([128, 512], FDT, tag="proj", name="vproj")
                    for d in range(2):
                        mm(
                            ps[:, :],
                            wv_sb[d][:, cb * 256 + c * 128:
                                     cb * 256 + c * 128 + 128],
                            xt_sb[d][:, :],
                            start=(d == 0), stop=(d == 1),
                        )
                    vT_eps[c].append(
                        spread(vT[c], ps, cb,
                               bvp_sb[:, cb * 2 + c:cb * 2 + c + 1], "dve"))

            tc.strict_bb_all_engine_barrier()
            # rotate V to key-major partitions: v_st[hl*8+t] = V[s' block t]^T
            v_tr = {}
            for hl in range(HPC):
                for t in range(8):
                    for c in range(2):
                        tr = nc.sync.dma_start_transpose(
                            out=v_st[hl * 8 + t][c][:, :],
                            in_=vT[c][:, hl * 1024 + t * 128:
                                      hl * 1024 + t * 128 + 128],
                        )
                        for ep in vT_eps[c]:
                            _dep(tr, ep, "xpose after vT writes")
                        v_tr[(hl, t, c)] = tr

            # output of attention, normalized, s'-major cols
            oh = [wqpool.tile([128, 4096], RDT, tag="w4k", name=f"oh{c}")
                  for c in range(2)]
            wot_sb = load(wqpool, "wot", wot[:, :], [128, 4096], dt=RDT,
                          tag="w4k")

            # ---- P3: attention per head; causal skip in s' order ----
            for hl in range(HPC):
                for qj in range(2):
                    last = 4 * qj + 3
                    po = [ps_o.tile([128, 512], FDT, tag="po", name=f"po{c}",
                                    bufs=3) for c in range(2)]
                    se = ps_se.tile([128, 512], FDT, tag="se", name="se",
                                    bufs=2)
                    for ki in range(last + 1):
                        dlt = ki - 4 * qj
                        off = 128 * dlt if dlt > 0 else 0
                        sp = ps_a.tile([128, 512], FDT, tag="proj",
                                       name="score")
                        for c in range(2):
                            mm(
                                sp[:, off:],
                                kt[c][:, hl * 1024 + ki * 128:
                                      hl * 1024 + ki * 128 + 128],
                                qt[c][:, hl * 1024 + qj * 512 + off:
                                      hl * 1024 + qj * 512 + 512],
                                start=(c == 0), stop=(c == 1),
                            )
                        pt = wpool.tile([128, 512], BDT, tag="pt", name="pt",
                                        bufs=6)
                        nc.scalar.activation(
                            pt[:, off:], sp[:, off:],
                            mybir.ActivationFunctionType.Exp,
                        )
                        if dlt >= 0:
                            if off:
                                nc.gpsimd.memset(pt[:, :off], 0.0)
                            # zero the causally-masked entries (0/1 mask)
                            nc.gpsimd.tensor_mul(
                                out=pt[:, off:], in0=pt[:, off:],
                                in1=mask_sb[:, :512 - off],
                            )
                        for c in range(2):
                            pv = mm(
                                po[c][:, :],
                                v_st[hl * 8 + ki][c][:, :],
                                pt[:, :],
                                start=(ki == 0), stop=(ki == last),
                            )
                            if (hl, ki, c) in v_tr:
                                _dep(pv, v_tr.pop((hl, ki, c)),
                                     "PV after V xpose")
                        mm(
                            se[:, :], ones_c[:, :], pt[:, :],
                            start=(ki == 0), stop=(ki == last),
                        )
                    rc = wpool.tile([128, 512], FDT, tag="rc", name="rc",
                                    bufs=3)
                    nc.vector.reciprocal(out=rc[:, :], in_=se[:, :])
                    for c in range(2):
                        nc.vector.tensor_mul(
                            out=oh[c][:, hl * 1024 + qj * 512:
                                      hl * 1024 + qj * 512 + 512],
                            in0=po[c][:, :], in1=rc[:, :],
                        )

            # ---- P4: output projection per head (y rows are per-head!) ----
            for hl in range(HPC):
                yp = ps_a.tile([128, 512], FDT, tag="proj", name="yproj")
                for cb in range(8):
                    for c in range(2):
                        j = 2 * cb + c
                        ohv = oh[c].rearrange("p (h r e) -> p h r e",
                                              h=4, e=8)
                        mm(
                            yp[:, 0:256],
                            ohv[:, hl:hl + 1, :, cb:cb + 1],
                            wot_sb[:, j * 256:(j + 1) * 256],
                            start=(j == 0), stop=(j == 15),
                        )
                ys = wpool.tile([128, 256], FDT, tag="ys", name="ys")
                nc.vector.tensor_copy(out=ys[:, :], in_=yp[:, 0:256])
                nc.sync.dma_start(
                    out=y[hl * 128:(hl + 1) * 128, :], in_=ys[:, :]
                )
    nc.finalize()
    return nc


def _prep_inputs(x, w_attn, b_attn, w_out):
    bf16 = ml_dtypes.bfloat16
    # single triangular mask: live col suffix j' of a diagonal tile obeys
    # (key partition p) <= j'
    rk = np.arange(128)
    rq = np.arange(512)
    mask_arr = (rk[:, None] <= rq[None, :]).astype(bf16)  # 1.0 live, 0.0 dead

    wqk = np.ascontiguousarray(
        np.concatenate([w_attn[0:2048] / 16.0, w_attn[2048:4096]]).T
    )  # (256, 4096)
    wvt = np.ascontiguousarray(w_attn[4096:6144].T)  # (256, 2048)
    bqk_arr = np.ascontiguousarray(
        np.concatenate([b_attn[0:2048] / 16.0, b_attn[2048:4096]])
        .reshape(32, 128).T
    )  # (128, 32)
    bvp_arr = np.ascontiguousarray(
        b_attn[4096:6144].reshape(16, 128).T
    ).astype(np.float32)  # (128, 16): col cb*2+c, partition dd
    wot_arr = np.ascontiguousarray(
        w_out.T.reshape(16, 128, 256).transpose(1, 0, 2).reshape(128, 4096)
    )

    in_maps = []
    for c in range(NCORES):
        b, g = divmod(c, 2)
        xt = np.ascontiguousarray(x[b, 512 * g:512 * (g + 1)].T)
        in_maps.append({
            "xt0": np.ascontiguousarray(xt[:128]),
            "xt1": np.ascontiguousarray(xt[128:]),
            "wqk0": np.ascontiguousarray(wqk[:128]),
            "wqk1": np.ascontiguousarray(wqk[128:]),
            "wv0": np.ascontiguousarray(wvt[:128]),
            "wv1": np.ascontiguousarray(wvt[128:]),
            "bqk": bqk_arr.astype(np.float32),
            "bvp": bvp_arr,
            "mask": mask_arr,
            "wot": wot_arr.astype(np.float32),
            "onc": np.ones((128, 128), bf16),
        })
    return in_maps


def kernel(x, w_attn, b_attn, w_out, b_out):
    x = np.asarray(x, dtype=np.float32)
    w_attn = np.asarray(w_attn, dtype=np.float32)
    b_attn = np.asarray(b_attn, dtype=np.float32)
    w_out = np.asarray(w_out, dtype=np.float32)
    b_out = np.asarray(b_out, dtype=np.float32)

    if "nc" not in _cache:
        _cache["nc"] = _build()
    nc = _cache["nc"]

    in_maps = _prep_inputs(x, w_attn, b_attn, w_out)
    res = run_bass_kernel_spmd(nc, in_maps, list(range(NCORES))).results

    out = np.empty((BS, SQL, EDIM), dtype=np.float32)
    for c in range(NCORES):
        b, g = divmod(c, 2)
        out[b, 512 * g:512 * (g + 1)] = res[c]["y"]
    out += b_out
    return out
